# revision 75
# baseline (speedup 1.0000x reference)
"""Trainium2 Bass kernel for DomainClassMixAugmentation.

Math: the four channel masks (cs&ds, cs&di, cg&ds, cg&di) partition the
(b, c) plane, so the whole module collapses to

    out[b] = A[b,c] * x[b] + Bs[b,c] * x[same_idx[b]] + Bd[b,c] * x[diff_idx[b]]

with per-(sample, channel) scalar coefficients

    A  = s0 where cs&ds, s1 where cg&ds, 1 elsewhere
    Bs = (1-s0) * (cs&ds)[same_idx]
    Bd = (1-s1) * (cg&ds)[diff_idx]

Two device launches (TimelineSim ~43.9us + ~18.0us; the 2e-2 tolerance
leaves ~3x margin at the measured 6.3e-3; the quantile masks are
protected separately by the host's banded exact refinement):

  A) importance reduce, channel-sharded: core k owns channels
     [32k, 32k+32) x all samples x full HW=3136 (partition =
     4-sample x 32-channel block, free = spatial), so one fused op
     reduces a [128, 3136] slab -- 8x the free dim of a spatially
     sharded op, amortizing per-op init/accum overhead. All three
     tensors stream as fp8/e4m3 (9.6MB/core, ~26.9us; fp16 x would be
     DMA-bound and the fp8 accumulation error, measured 2.9e-3, stays
     under the 5e-3 refinement band). Engine schedule in _sched_A:
     DVE fused STT multiply+reduce (~37.0us, the critical path; the
     fp8 operands lock DVE to 1x = 1.0417 ns/elem), Pool products
     (1.98 ns/elem at 0.42 efficiency) over a per-chunk dg slice with
     Act activation-accum reduces; Pool's slice is sized to its
     arrival chain (~32us) and tapers at the tail.

  B) sparse apply: the output differs from x only on the active slabs
     R[b] = ds[b] | m1[same[b]] | m3[diff[b]] (~37% of (b,c); ds is
     exactly 51/256 channels by the 0.8-quantile). Untouched slabs are
     exact f32 copies of x on the host; every slab where the module
     does arithmetic is computed on device. Active slabs are packed
     into 128-row tiles by first-fit-decreasing bin packing of the
     channel groups (each <= B=32 rows; channel order is free, and FFD
     reaches the ideal tile count where sequential packing wastes ~1
     tile), so every source of an out-slab sits in the SAME tile and
     out-tile t is ONE [128,128] u8 matmul
     W[t] @ x-tile(t) on PE -- the data-dependent diag+gather
     structure lives entirely in host-built W. Spatially sharded
     (SP=392/core); xs/wd/ot are single resident SBUF tiles; W+qs
     stream first, then x chunks with per-chunk PSUM->u8 quantizing
     copies (per-slab inverse-bound scale, alternating Act/DVE) and
     out-DMAs on alternating queues. ~4.2MB/core total DMA.
     Tile-count ladder SPARSE_NS + dense matmul fallback (_build_
     apply_nc) if a run's masks need more tiles.

Host in between: sum per-core partials, take the two per-sample
quantiles, and exactly recompute (from the original f32 tensors) every
channel whose device-accumulated importance lies within a guard band
of a threshold/rank boundary -- so the masks match the reference's f32
decisions bit-for-bit. The u8 outputs are dequantized on the host with
the same per-slab scales the device applied.

Measured dead ends (do not revisit without new primitives): fused
single launch (masks from device sums flip vs the f32 reference; the
host refinement step is load-bearing); PE-based reduction (diag
extraction costs a full elementwise pass); spatial subsampling of the
reduce (estimator noise ~ signal spread, the band would cover every
channel); Act-upconvert C-path to unlock DVE 2x products (+7us naive,
still +2.8us with P-first emission and deeper pools: Act's in-order
queue cannot absorb 5.6us convert blobs between its Pool-reduces, and
the LP's idle-Act capacity is not schedulable); tensor_tensor_
reduce (2 sub-ops, same cost as STT); tensor_tensor_scan as a 2x
product (bypass,mult degenerates to an elementwise product but runs
1x like TT -- only plain tensor_scalar has the dtype-independent
2x_2p mode, and it cannot multiply two tensors); PE ones-matmul reduction over a
spatial-major layout (padding + per-op inits + losing Act's reduce
contribution eat the theoretical 34.3us joint-product floor, net ~-1);
note every DMA-dependent start also pays the 900ns DMA-completion
semaphore (SEM_PROP_DMA_OVERHEAD_NS) -- the ~4.2us DVE ramp is floor; per-group tail out-DMAs and
fine chunking (SP-queue issue is ~650ns per DMA instruction); Pool-
queue DMAs (SWDGE runs on the Pool engine itself, ~1us each);
fp8 x in the apply (2.5e-2 max-abs error, too close to the gate).
"""

import hashlib
import os
import time

import numpy as np

import concourse.bacc as bacc
import concourse.bass as bass
import concourse.mybir as mybir
import concourse.tile as tile
from concourse import bass2jax

_NEFF_CACHE_DIR = os.path.join(
    os.path.expanduser("~"), ".cache", "bass_neff_cache"
)


def _install_cached_hook():
    """bass2jax's neuronx_cc hook recompiles the NEFF (minutes) on every
    fresh process; wrap it with a content-addressed disk cache."""
    bass2jax.install_neuronx_cc_hook()
    try:
        import libneuronxla
    except ImportError:
        return
    if getattr(libneuronxla, "_ant_disk_cache", False):
        return
    orig = libneuronxla.neuronx_cc
    os.makedirs(_NEFF_CACHE_DIR, exist_ok=True)

    def canonical(code):
        # the raw HLO embeds per-op source_file/source_line metadata, so the
        # same kernel run from a different path/line offset would re-key;
        # strip it before hashing.
        try:
            import libneuronxla.proto.hlo_pb2 as hlo_pb2

            p = hlo_pb2.HloModuleProto.FromString(bytes(code))
            for field in ("stack_frame_index",):
                try:
                    p.ClearField(field)
                except ValueError:
                    pass
            for comp in p.computations:
                for ins in comp.instructions:
                    ins.ClearField("metadata")
            return p.SerializeToString(deterministic=True)
        except Exception:
            return bytes(code)

    def cached(code, code_format, platform_version, file_prefix):
        key = hashlib.sha256(
            b"|".join(
                [canonical(code), bytes(code_format), str(platform_version).encode()]
            )
        ).hexdigest()
        path = os.path.join(_NEFF_CACHE_DIR, key + ".bin")
        if os.path.exists(path):
            with open(path, "rb") as f:
                return 0, f.read()
        ret, data = orig(code, code_format, platform_version, file_prefix)
        if ret == 0 and isinstance(data, bytes) and len(data) > 0:
            tmp = path + f".tmp{os.getpid()}"
            with open(tmp, "wb") as f:
                f.write(data)
            os.replace(tmp, path)
        return ret, data

    libneuronxla.neuronx_cc = cached
    libneuronxla._ant_disk_cache = True

B, C, H, W = 32, 256, 56, 56
NCORES = 8
SH = H // NCORES          # 7 rows of H per core
SP = SH * W               # 392 spatial elements per core per (b, c)
HALVES = C // 128         # 2 partition blocks of channels
NT = B * HALVES           # 64 accumulator columns (j = b*2 + h)
NTC = C // 4              # 64 channel-groups of 4; one matmul each
F32 = mybir.dt.float32
F16 = mybir.dt.float16
AOP = mybir.AluOpType

# ---- Launch A op schedule (channel-sharded) ---------------------------------
# Core k owns channels [32k, 32k+32) for ALL samples and ALL spatial HW=3136.
# Partition p = (b % 4)*32 + (c % 32); block g = b // 4 selects 4 samples.
# A reduction op covers one (grad, g, spatial-span) piece: product engines are
# DVE (fused scalar_tensor_tensor + accum, 1.0417 ns/elem at 1x -- the fp8
# operands block 2x mode) and Pool (tensor_tensor at 0.42 efficiency,
# ~1.98 ns/elem) whose product tile is then reduced by an Act activation-accum
# (0.833 ns/elem + ~370ns fixed). The long [128, 3136] free dim amortizes the
# per-op init/accum overheads that dominated the old 392-wide spatial shard.
HWFULL = H * W            # 3136 spatial elements per (b, c) on one core
NBG = B // 4              # 8 four-sample blocks
CPC = C // NCORES         # 32 channels per core


def _sched_A():
    """Op list: (grad, g, s_lo, s_hi, engine, col). Engines: 'S' = DVE fused,
    'P' = Pool product + Act reduce ('C' also exists in the builder: Act
    upconverts + DVE 2x product -- measured a net loss, unused).

    With the all-fp8 stream a 2-block chunk arrives every ~3.34us, faster
    than the engines drain it, so both DVE (~37.0us busy) and Pool
    (~30.8us + Act chain) run saturated from their first arrival; the
    schedule balances their END times. Pool takes a 2304-elem slice of
    each dg block (its arrival+Act chain caps it), tapering on the last
    two chunks so the Act reduce never becomes the tail; DVE takes all
    cg blocks + the dg remainders. Act ~16us."""
    ops = []
    col = 0
    H1 = HWFULL // 2
    PCUT = 2304               # Pool's dg slice on the middle chunks

    def add(grad, g, lo, hi, eng):
        nonlocal col
        ops.append((grad, g, lo, hi, eng, col))
        col += 1

    for g in range(NBG):
        if g == 0:
            # ramp chunk streams as halves; first op after x-h0+cg-h0 land
            add(0, g, 0, H1, "S")
            add(0, g, H1, HWFULL, "S")
            add(1, g, 0, H1, "S")
            add(1, g, H1, HWFULL, "P")
        else:
            pcut = PCUT if g <= 5 else (1792 if g == 6 else 1280)
            add(0, g, 0, HWFULL, "S")
            if g == NBG - 1:
                # split the final Pool piece so its first Act reduce runs
                # while the second half is still multiplying: the flush
                # waits on the LAST Act reduce, which now covers half the
                # elements
                add(1, g, 0, pcut // 2, "P")
                add(1, g, pcut // 2, pcut, "P")
            else:
                add(1, g, 0, pcut, "P")
            add(1, g, pcut, HWFULL, "S")
    return ops


_OPS_A = _sched_A()
NCOLS_A = len(_OPS_A)
# columns belonging to the tail block (flushed in the final small DMA)
_TAIL_COLS_A = [c for (grad, g, lo, hi, eng, c) in _OPS_A if g == NBG - 1]

# Launch B channel-group chunk sizes (sum = NTC). The in-DMAs are all
# requested early (SP queue), so the DMA pool runs every input first and
# the quantized outputs drain afterwards; 8 even chunks keep compute ~2
# chunks ahead of the pool so the output drain never waits on compute.
CHUNKS_B = [12, 12, 12, 12, 8, 4, 4]

# Guard band for the device-accumulated importance means (units of the
# mean, i.e. sum/3136). With fp8/e4m3 x and gradients the measured max
# error over all 8192 channels is ~2.9e-3; 5e-3 covers it with margin,
# and every channel inside the band is recomputed exactly on the host,
# so mask decisions match the f32 reference.
BAND = 5e-3

_CACHE: dict = {}


def _build_reduce_nc():
    """Launch A: per-core sums of x*cg and x*dg, channel-sharded.

    DRAM layout per tensor: [128, NBG*HWFULL] with [p, g*HWFULL + s] =
    tensor[g*4 + p//32, 32*core + p%32, s]. One DMA per (tensor, block)
    for the middle blocks; head/tail blocks stream as spatial quarters
    (head quarters spread over the four issue queues so the ramp isn't
    serialized on one queue's ~650ns/DMA issue cost).
    Output imp [128, NCOLS_A] f32: one column per _OPS_A entry.
    """
    nc = bacc.Bacc("TRN2", target_bir_lowering=False, debug=False)
    F8 = mybir.dt.float8e4
    x = nc.dram_tensor("x", [128, NBG * HWFULL], F8, kind="ExternalInput").ap()
    cg = nc.dram_tensor("cg", [128, NBG * HWFULL], F8, kind="ExternalInput").ap()
    dg = nc.dram_tensor("dg", [128, NBG * HWFULL], F8, kind="ExternalInput").ap()
    imp = nc.dram_tensor("imp", [128, NCOLS_A], F32, kind="ExternalOutput").ap()

    ops_by_g = {}
    for op in _OPS_A:
        ops_by_g.setdefault(op[1], []).append(op)

    with tile.TileContext(nc) as tc:
        with (
            tc.tile_pool(name="io", bufs=8) as io,
            tc.tile_pool(name="sd", bufs=2) as sd,
            tc.tile_pool(name="pq", bufs=2) as pq,
            tc.tile_pool(name="cv", bufs=2) as cv,
            tc.tile_pool(name="sa", bufs=2) as sa,
            tc.tile_pool(name="acc", bufs=1) as acc,
        ):
            big = acc.tile([128, NCOLS_A], F32, tag="big", name="big")
            # NOTE: never issue DMAs from nc.gpsimd -- Pool-queue DMAs run
            # SWDGE descriptor generation on the Pool engine itself (~1us
            # per DMA of engine time). SP/Act/DVE queues use the HWDGE.
            queues = [nc.sync, nc.scalar]
            for g in range(NBG):
                split = g == 0
                tiles = {}
                srcs = (("xt", x, F8), ("ct", cg, F8), ("gt", dg, F8))
                for name, src, dt_ in srcs:
                    tiles[name] = io.tile(
                        [128, HWFULL], dt_, tag=name, name=f"{name}{g}"
                    )
                if split:
                    # half DMAs, interleaved x/cg/dg so each half's ops can
                    # fire as soon as its inputs land; spread across the two
                    # HWDGE issue queues (SP + Act)
                    H1 = HWFULL // 2
                    for q in range(2):
                        for qi, (name, src, dt_) in enumerate(srcs):
                            queue = queues[qi % 2]
                            queue.dma_start(
                                tiles[name][:, q * H1:(q + 1) * H1],
                                src[:, g * HWFULL + q * H1:
                                    g * HWFULL + (q + 1) * H1],
                            )
                else:
                    for name, src, dt_ in srcs:
                        nc.sync.dma_start(
                            tiles[name][:],
                            src[:, g * HWFULL:(g + 1) * HWFULL],
                        )
                for grad, _, lo, hi, eng, col in ops_by_g[g]:
                    gt = tiles["ct"] if grad == 0 else tiles["gt"]
                    if eng == "S":
                        s1 = sd.tile([128, hi - lo], F16, tag=f"s{hi - lo}")
                        nc.vector.scalar_tensor_tensor(
                            out=s1[:], in0=tiles["xt"][:, lo:hi], scalar=0.0,
                            in1=gt[:, lo:hi], op0=AOP.bypass, op1=AOP.mult,
                            accum_out=big[:, col:col + 1],
                        )
                    elif eng == "C":
                        # Act upconverts both fp8 operands to fp16; DVE then
                        # multiplies in 2x mode; Act reduces the product
                        xc = cv.tile([128, hi - lo], F16, tag=f"x{hi - lo}")
                        nc.scalar.activation(
                            out=xc[:], in_=tiles["xt"][:, lo:hi],
                            func=mybir.ActivationFunctionType.Copy,
                        )
                        gc = cv.tile([128, hi - lo], F16, tag=f"g{hi - lo}")
                        nc.scalar.activation(
                            out=gc[:], in_=gt[:, lo:hi],
                            func=mybir.ActivationFunctionType.Copy,
                        )
                        p = pq.tile([128, hi - lo], F16, tag=f"q{hi - lo}")
                        nc.vector.tensor_tensor(
                            out=p[:], in0=xc[:], in1=gc[:], op=AOP.mult,
                        )
                        s2 = sa.tile([128, hi - lo], F16, tag=f"a{hi - lo}")
                        nc.scalar.activation(
                            out=s2[:], in_=p[:],
                            func=mybir.ActivationFunctionType.Copy,
                            accum_out=big[:, col:col + 1],
                        )
                    else:
                        p = pq.tile([128, hi - lo], F16, tag=f"q{hi - lo}")
                        nc.gpsimd.tensor_tensor(
                            out=p[:], in0=tiles["xt"][:, lo:hi],
                            in1=gt[:, lo:hi], op=AOP.mult,
                        )
                        s2 = sa.tile([128, hi - lo], F16, tag=f"a{hi - lo}")
                        nc.scalar.activation(
                            out=s2[:], in_=p[:],
                            func=mybir.ActivationFunctionType.Copy,
                            accum_out=big[:, col:col + 1],
                        )
            # Flush in two pieces: all non-tail columns fire while the tail
            # block computes; the final DMA moves only the tail columns.
            cut = _TAIL_COLS_A[0]
            nc.sync.dma_start(imp[:, 0:cut], big[:, 0:cut])
            nc.sync.dma_start(imp[:, cut:NCOLS_A], big[:, cut:NCOLS_A])
    nc.compile()
    return nc


def _build_apply_nc():
    """Launch B: out[b, c, :] = sum_j W[c, j, b] * x[j, c, :] via PE.

    W (host-built) folds the diagonal A term and the same/diff
    one-hot gather terms into one [B, B] matrix per channel, packed 4
    channels per [128, 128] lhsT (lane-interleaved: k = j*4+cc,
    m = b*4+cc).  x is host-packed to the matching [128, NTC*SP] fp16
    layout (row j*4+cc, col q*SP+s).  fp16 matmuls run at 1 cyc/row.
    W travels as uint8 fixed-point (every coefficient lies in [0, 1],
    so absolute quantization error is <= 0.5/255 + fp16 eps, ~50x under
    the output tolerance; 0 and 1 encode exactly) and one DVE scaled
    copy per chunk dequantizes it to fp16, halving the W DMA bytes.
    The output is uint8 fixed-point: the PSUM->SBUF copies scale by a
    per-partition inverse bound (runtime input qs, since the bound
    depends on the masks) and bias 128; the host dequantizes. Max
    quantization error = bound/254 ~ 4.4e-2 absolute = ~6e-3 relative
    (gate 2e-2). This halves the out-DMA bytes vs fp16.
    Loads stream on the SP queue; PSUM->SBUF quantizing copies
    alternate between Act and DVE; output DMAs issue from the Act queue.
    """
    nc = bacc.Bacc("TRN2", target_bir_lowering=False, debug=False)
    U8 = mybir.dt.uint8
    xq = nc.dram_tensor("xq", [128, NTC * SP], F16, kind="ExternalInput").ap()
    w = nc.dram_tensor("w", [128, NTC * 128], U8, kind="ExternalInput").ap()
    qs = nc.dram_tensor("qs", [128, 1], F32, kind="ExternalInput").ap()
    out = nc.dram_tensor("out", [128, NTC * SP], U8, kind="ExternalOutput").ap()

    with tile.TileContext(nc) as tc:
        with (
            tc.tile_pool(name="qsp", bufs=1) as qsp,
            tc.tile_pool(name="wp", bufs=3) as wp,
            tc.tile_pool(name="wf", bufs=3) as wf,
            tc.tile_pool(name="io", bufs=4) as io,
            tc.tile_pool(name="ps", bufs=8, space="PSUM") as ps,
            tc.tile_pool(name="ob", bufs=8) as ob,
        ):
            qst = qsp.tile([128, 1], F32, tag="qst", name="qst")
            q0 = 0
            for ci, qn in enumerate(CHUNKS_B):
                is_last = ci == len(CHUNKS_B) - 1
                wt = wp.tile([128, qn * 128], U8, tag="wt")
                nc.sync.dma_start(wt[:], w[:, q0 * 128:(q0 + qn) * 128])
                if ci == 0:
                    nc.sync.dma_start(qst[:], qs)
                wd = wf.tile([128, qn * 128], F16, tag="wd")
                nc.vector.tensor_scalar_mul(wd[:], wt[:], 1.0 / 255.0)
                rt = io.tile([128, qn * SP], F16, tag="rt")
                nc.sync.dma_start(rt[:], xq[:, q0 * SP:(q0 + qn) * SP])
                ot = ob.tile([128, qn * SP], U8, tag="ot")
                for lq in range(qn):
                    pt = ps.tile([128, SP], F32, tag="pt")
                    nc.tensor.matmul(
                        pt[:],
                        lhsT=wd[:, lq * 128:(lq + 1) * 128],
                        rhs=rt[:, lq * SP:(lq + 1) * SP],
                        start=True, stop=True,
                    )
                    osl = ot[:, lq * SP:(lq + 1) * SP]
                    if lq % 2 == 0:
                        nc.scalar.activation(
                            out=osl, in_=pt[:],
                            func=mybir.ActivationFunctionType.Copy,
                            scale=qst[:, 0:1], bias=128.0,
                        )
                    else:
                        nc.vector.tensor_scalar(
                            out=osl, in0=pt[:], scalar1=qst[:, 0:1],
                            scalar2=128.0, op0=AOP.mult, op1=AOP.add,
                        )
                    # last chunk: flush its first half early so the final
                    # out-DMA only moves the second half
                    if is_last and lq == qn // 2 - 1:
                        nc.scalar.dma_start(
                            out[:, q0 * SP:(q0 + qn // 2) * SP],
                            ot[:, :qn // 2 * SP],
                        )
                if is_last:
                    nc.scalar.dma_start(
                        out[:, (q0 + qn // 2) * SP:(q0 + qn) * SP],
                        ot[:, qn // 2 * SP:],
                    )
                else:
                    nc.scalar.dma_start(out[:, q0 * SP:(q0 + qn) * SP], ot[:])
                q0 += qn
    nc.compile()
    return nc


# Sparse apply: tile-count ladder (tile-aligned universe slabs / 128). The
# active-slab universe on B=32/C=256 data is ~3030 slabs in 256 channel
# groups; aligned packing (no group straddles a 128-tile boundary) needs
# ~25 tiles. Larger Ns compile lazily only if a run's masks need them;
# beyond the ladder the dense apply kernel is the fallback.
SPARSE_NS = [24, 25, 27, 30, 33]


def _build_sparse_apply_nc(n_tiles):
    """Launch B (sparse): only the ~37% of (sample, channel) slabs where the
    module's output differs from x are processed; untouched slabs pass
    through on the host as exact f32 copies of x.

    Universe: the active slabs (c, j), ordered channel-major and packed so
    that no channel's slab group straddles a 128-tile boundary. All sources
    of an out-slab share its channel, so out-tile t = W[t] @ x-tile(t) with
    ONE host-filled [128,128] u8 block per tile -- the data-dependent gather
    structure lives entirely in W. A per-partition-scaled quantizing copy
    (alternating Act/DVE) emits u8; the host dequantizes and scatters.

    xs / wd / ot are single resident SBUF tiles (~31KB/partition total), so
    chunked DMAs just fill slices and no pool cycling is needed.
    """
    nc = bacc.Bacc("TRN2", target_bir_lowering=False, debug=False)
    U8 = mybir.dt.uint8
    N = n_tiles
    xs = nc.dram_tensor("xs", [128, N * SP], F16, kind="ExternalInput").ap()
    w = nc.dram_tensor("w", [128, N * 128], U8, kind="ExternalInput").ap()
    qs = nc.dram_tensor("qs", [128, N], F32, kind="ExternalInput").ap()
    out = nc.dram_tensor("out", [128, N * SP], U8, kind="ExternalOutput").ap()

    # tile chunks: small head for ramp, small tail for drain
    chunks = [2]
    while sum(chunks) + 3 < N:
        chunks.append(min(4, N - 3 - sum(chunks)))
    chunks += [2, 1]
    assert sum(chunks) == N

    with tile.TileContext(nc) as tc:
        with (
            tc.tile_pool(name="res", bufs=1) as res,
            tc.tile_pool(name="ps", bufs=8, space="PSUM") as ps,
        ):
            xst = res.tile([128, N * SP], F16, tag="xst", name="xst")
            wt = res.tile([128, N * 128], U8, tag="wt", name="wt")
            wd = res.tile([128, N * 128], F16, tag="wd", name="wd")
            qst = res.tile([128, N], F32, tag="qst", name="qst")
            ot = res.tile([128, N * SP], U8, tag="ot", name="ot")
            # W is tiny (N*16KB u8): stream it all up front (+ qs) so the x
            # stream is gated by nothing and the last tile's x lands early
            nc.sync.dma_start(wt[:, 0:2 * 128], w[:, 0:2 * 128])
            nc.sync.dma_start(qst[:], qs)
            nc.sync.dma_start(wt[:, 2 * 128:], w[:, 2 * 128:])
            nc.vector.tensor_scalar_mul(wd[:, 0:2 * 128], wt[:, 0:2 * 128],
                                        1.0 / 255.0)
            nc.vector.tensor_scalar_mul(wd[:, 2 * 128:], wt[:, 2 * 128:],
                                        1.0 / 255.0)
            t0 = 0
            for ci, tn in enumerate(chunks):
                xsl = slice(t0 * SP, (t0 + tn) * SP)
                nc.sync.dma_start(xst[:, xsl], xs[:, xsl])
                # emit this chunk's compute right here so the out-DMAs can
                # interleave with later in-chunks on the DMA pool
                for ti, t in enumerate(range(t0, t0 + tn)):
                    pt = ps.tile([128, SP], F32, tag="pt")
                    nc.tensor.matmul(
                        pt[:],
                        lhsT=wd[:, t * 128:(t + 1) * 128],
                        rhs=xst[:, t * SP:(t + 1) * SP],
                        start=True, stop=True,
                    )
                    osl = ot[:, t * SP:(t + 1) * SP]
                    # Act takes the chunk's first tiles, DVE the last ones:
                    # each chunk's out-DMA then waits on the faster engine,
                    # and the two quantize queues stay balanced
                    if ti < tn // 2:
                        nc.scalar.activation(
                            out=osl, in_=pt[:],
                            func=mybir.ActivationFunctionType.Copy,
                            scale=qst[:, t:t + 1], bias=128.0,
                        )
                    else:
                        nc.vector.tensor_scalar(
                            out=osl, in0=pt[:], scalar1=qst[:, t:t + 1],
                            scalar2=128.0, op0=AOP.mult, op1=AOP.add,
                        )
                oq = nc.sync if ci % 2 == 0 else nc.scalar
                oq.dma_start(
                    out[:, t0 * SP:(t0 + tn) * SP],
                    ot[:, t0 * SP:(t0 + tn) * SP],
                )
                t0 += tn
    nc.compile()
    return nc


def _get_nc(key):
    if key not in _CACHE:
        if key == "reduce":
            _CACHE[key] = _build_reduce_nc()
        elif key == "apply":
            _CACHE[key] = _build_apply_nc()
        elif key.startswith("sparse"):
            _CACHE[key] = _build_sparse_apply_nc(int(key[6:]))
    return _CACHE[key]


class _Runner:
    """Cached PJRT runner for a compiled Bass module (8-core SPMD).

    Mirrors bass2jax.run_bass_via_pjrt's multi-core path, but keeps the
    jitted executable (so repeat calls don't re-trace), accepts
    pre-uploaded device arrays, and materialises the donated output
    buffers on device instead of uploading host zeros.
    """

    def __init__(self, nc, n_cores=NCORES):
        import jax
        import jax.numpy as jnp
        from jax.experimental.shard_map import shard_map
        from jax.sharding import Mesh, NamedSharding, PartitionSpec

        _install_cached_hook()
        self.n_cores = n_cores
        pid_name = nc.partition_id_tensor.name if nc.partition_id_tensor else None
        in_names, out_names, out_avals = [], [], []
        for alloc in nc.m.functions[0].allocations:
            if not isinstance(alloc, mybir.MemoryLocationSet):
                continue
            name = alloc.memorylocations[0].name
            if alloc.kind == "ExternalInput":
                if name != pid_name:
                    in_names.append(name)
            elif alloc.kind == "ExternalOutput":
                out_names.append(name)
                out_avals.append(
                    jax.core.ShapedArray(
                        tuple(alloc.tensor_shape), mybir.dt.np(alloc.dtype)
                    )
                )
        self.in_names = in_names
        self.out_names = out_names
        self.out_avals = out_avals
        n_params = len(in_names)
        bind_names = list(in_names) + list(out_names)
        if pid_name is not None:
            bind_names.append(pid_name)

        def _body(*args):
            operands = list(args)
            if pid_name is not None:
                operands.append(bass2jax.partition_id_tensor())
            return tuple(
                bass2jax._bass_exec_p.bind(
                    *operands,
                    out_avals=tuple(out_avals),
                    in_names=tuple(bind_names),
                    out_names=tuple(out_names),
                    lowering_input_output_aliases=(),
                    sim_require_finite=True,
                    sim_require_nnan=True,
                    nc=nc,
                )
            )

        mesh = Mesh(np.asarray(jax.devices()[:n_cores]), ("core",))
        self.sharding = NamedSharding(mesh, PartitionSpec("core"))
        n_outs = len(out_names)
        self._sharded = jax.jit(
            shard_map(
                _body,
                mesh=mesh,
                in_specs=(PartitionSpec("core"),) * (n_params + n_outs),
                out_specs=(PartitionSpec("core"),) * n_outs,
                check_rep=False,
            ),
            donate_argnums=tuple(range(n_params, n_params + n_outs)),
            keep_unused=True,
        )
        self._zeros = jax.jit(
            lambda: tuple(
                jnp.zeros((n_cores * a.shape[0], *a.shape[1:]), a.dtype)
                for a in out_avals
            ),
            out_shardings=tuple(self.sharding for _ in out_avals),
        )

    def put(self, per_core_arrays):
        """Upload a list of per-core np arrays as one sharded device array."""
        import jax

        return jax.device_put(np.concatenate(per_core_arrays, axis=0), self.sharding)

    def put_replicated(self, arr):
        import jax

        return jax.device_put(
            np.concatenate([arr] * self.n_cores, axis=0), self.sharding
        )

    def __call__(self, *device_args):
        """Run with device (or host) args in in_names order; returns jax arrays."""
        return self._sharded(*device_args, *self._zeros())


def _get_runner(key):
    rkey = key + "_runner"
    if rkey not in _CACHE:
        _CACHE[rkey] = _Runner(_get_nc(key))
    return _CACHE[rkey]


def _gather_partials(arr, gi):
    """[NCORES, 128, NCOLS_A] device partials for grad index gi -> [B, C].

    Cell (b, c) lives on core k = c // CPC, partition p = (b%4)*CPC + c%CPC,
    block g = b // 4; its value is the sum of the accum columns of the ops
    covering (gi, g)'s spatial pieces.
    """
    acc = np.zeros((NCORES, 128, NBG), dtype=np.float32)
    for grad, g, lo, hi, eng, col in _OPS_A:
        if grad == gi:
            acc[:, :, g] += arr[:, :, col]
    # [k, (b_sub, cl), g] -> [b = g*4 + b_sub, c = k*CPC + cl]
    out = (
        acc.reshape(NCORES, 4, CPC, NBG)
        .transpose(3, 1, 0, 2)
        .reshape(B, C)
    )
    return np.ascontiguousarray(out)


def _exact_mask(vdev, xs, gs, q, band):
    """Masks (val > q-quantile) matching the f32 reference bit-for-bit.

    vdev [B, C]: device-accumulated fp16-input means (error << band).
    xs/gs [B, C, HW]: the original f32 tensors. Channels whose device
    value lies within the guard band of a rank boundary or the threshold
    are recomputed exactly; everything else is decided from vdev (its
    error is < band, and it sits > band away from the threshold).
    """
    n = vdev.shape[1]
    qf = np.float32(q) * np.float32(n - 1)
    lo = int(np.floor(qf))
    hi = int(np.ceil(qf))
    hw_ = np.float32(qf - np.float32(lo))
    lw = np.float32(np.float32(1.0) - hw_)
    inv = np.float32(1.0) / np.float32(xs.shape[2])
    band = np.float32(band)
    mask = np.zeros(vdev.shape, dtype=bool)
    for b in range(vdev.shape[0]):
        v = vdev[b]
        sv = np.sort(v)
        lo_val, hi_val = sv[lo], sv[hi]
        lhs = np.float32(lo_val - 2 * band)
        rhs = np.float32(hi_val + 2 * band)
        cand = np.where((v >= lhs) & (v <= rhs))[0]
        ex = {
            int(c): np.float32(np.dot(xs[b, c], gs[b, c]) * inv) for c in cand
        }
        n_below = int(np.sum(v < lhs))
        exs = np.sort(np.asarray([ex[int(c)] for c in cand], dtype=np.float32))
        tlo = exs[lo - n_below]
        thi = exs[hi - n_below]
        thr = np.float32(tlo * lw + thi * hw_)
        vals = v.copy()
        for c, e in ex.items():
            vals[c] = e
        mask[b] = vals > thr
    return mask


def kernel(**inputs):
    x = np.asarray(inputs["x"], dtype=np.float32)
    cg = np.asarray(inputs["class_gradient"], dtype=np.float32)
    dg = np.asarray(inputs["domain_gradient"], dtype=np.float32)
    ms = np.asarray(inputs["mixup_strength"], dtype=np.float32)
    same_idx = np.asarray(inputs["same_idx"]).astype(np.int64)
    diff_idx = np.asarray(inputs["diff_idx"]).astype(np.int64)

    times = {}
    t0 = time.perf_counter()
    np_f8 = mybir.dt.np(mybir.dt.float8e4)
    x16 = x.astype(np.float16)
    cg8 = cg.astype(np_f8)
    dg8 = dg.astype(np_f8)

    # ---- launch A shards: core k gets channels [k*CPC, (k+1)*CPC) --------
    def a_shards(t):
        # [B, C, H, W] -> per-core [128, NBG*HWFULL] with
        # [(b%4)*CPC + c%CPC, (b//4)*HWFULL + s] = t[b, k*CPC + c%CPC, s]
        tf = t.reshape(B, C, HWFULL)
        out = []
        for k in range(NCORES):
            sl = tf[:, k * CPC:(k + 1) * CPC, :].reshape(NBG, 4, CPC, HWFULL)
            out.append(
                np.ascontiguousarray(sl.transpose(1, 2, 0, 3)).reshape(
                    128, NBG * HWFULL
                )
            )
        return out

    x8 = x.astype(np_f8)
    x_sl = a_shards(x8)
    cg_sl = a_shards(cg8)
    dg_sl = a_shards(dg8)
    times["prep"] = time.perf_counter() - t0

    # ---- launch A: partial importance sums -------------------------------
    ra = _get_runner("reduce")
    t0 = time.perf_counter()
    x_dev = ra.put(x_sl)
    cg_dev = ra.put(cg_sl)
    dg_dev = ra.put(dg_sl)
    times["upload_a"] = time.perf_counter() - t0
    t0 = time.perf_counter()
    for attempt in range(3):
        try:
            outs_a = ra(x_dev, cg_dev, dg_dev)
            partials = np.asarray(outs_a[0]).reshape(NCORES, 128, NCOLS_A)
            break
        except Exception:
            # transient NRT/axon exec failures happen; re-upload and retry
            if attempt == 2:
                raise
            time.sleep(2.0)
            x_dev = ra.put(x_sl)
            cg_dev = ra.put(cg_sl)
            dg_dev = ra.put(dg_sl)
    times["exec_a"] = time.perf_counter() - t0

    inv_n = np.float32(1.0) / np.float32(H * W)
    cim = _gather_partials(partials, 0) * inv_n
    dim = _gather_partials(partials, 1) * inv_n
    _CACHE["last_cim_dev"] = cim
    _CACHE["last_dim_dev"] = dim

    # ---- host: masks via banded exact refinement, coefficients, W --------
    t0 = time.perf_counter()
    xf = x.reshape(B, C, H * W)
    cs = _exact_mask(cim, xf, cg.reshape(B, C, H * W), 0.5, BAND)
    ds = _exact_mask(dim, xf, dg.reshape(B, C, H * W), 0.8, BAND)
    m1 = cs & ds          # class-salient & domain-salient
    m3 = (~cs) & ds       # class-generic & domain-salient

    s0 = ms[:, 0].astype(np.float32)[:, None]
    s1 = ms[:, 1].astype(np.float32)[:, None]
    one = np.float32(1.0)

    A = np.where(m1, s0, np.where(m3, s1, one)).astype(np.float32)
    Bs = np.where(m1[same_idx], one - s0, np.float32(0.0)).astype(np.float32)
    Bd = np.where(m3[diff_idx], one - s1, np.float32(0.0)).astype(np.float32)

    # active slabs: where the output can differ from x
    R = ds | m1[same_idx] | m3[diff_idx]
    # tile-aligned packing: first-fit-decreasing bin packing of the channel
    # groups (each <= B <= 128 slabs) into 128-slab tiles -- channel order
    # within the universe is free, and FFD reaches the ideal tile count
    # (sequential packing wastes ~1 tile to boundary padding)
    gsz = R.sum(axis=0).astype(np.int64)      # group size per channel
    starts = np.zeros(C, dtype=np.int64)
    bin_used = []                             # slabs used per tile
    for c in np.argsort(-gsz, kind="stable"):
        g = int(gsz[c])
        if g == 0:
            continue
        for i in range(len(bin_used)):
            if bin_used[i] + g <= 128:
                starts[c] = i * 128 + bin_used[i]
                bin_used[i] += g
                break
        else:
            starts[c] = len(bin_used) * 128
            bin_used.append(g)
    n_need = len(bin_used)
    n_tiles = next((n for n in SPARSE_NS if n >= n_need), None)
    mx = np.abs(x16).max(axis=(2, 3)).astype(np.float32)  # [B, C]

    if n_tiles is not None:
        # ---- sparse apply: aligned channel-major active-slab universe ----
        N = n_tiles
        cj = np.argwhere(R.T)                      # (c, j) lexicographic
        c_u = cj[:, 0]
        j_u = cj[:, 1]
        NU = len(c_u)
        # position of slab (c, j): starts[c] + rank of j within its group
        within = np.arange(NU) - np.searchsorted(c_u, c_u)
        pos_u = starts[c_u] + within               # universe position per slab
        pos = np.full((C, B), -1, dtype=np.int64)
        pos[c_u, j_u] = pos_u
        t_u = pos_u // 128
        p_u = pos_u % 128
        W4 = np.zeros((N, 128, 128), dtype=np.float32)
        W4[t_u, p_u, p_u] += A[j_u, c_u]
        for coefs, idx in ((Bs, same_idx), (Bd, diff_idx)):
            cf = coefs[j_u, c_u]
            nz = np.nonzero(cf)[0]
            v = pos[c_u[nz], idx[j_u[nz]]]
            assert (v >= 0).all() and (v // 128 == t_u[nz]).all()
            np.add.at(W4, (t_u[nz], v % 128, p_u[nz]), cf[nz])
        Wt_s = np.rint(
            np.clip(
                np.ascontiguousarray(
                    W4.transpose(1, 0, 2).reshape(128, N * 128)
                ),
                0.0, 1.0,
            ) * np.float32(255.0)
        ).astype(np.uint8)
        # per-slab output bound -> inverse scale [128, N]
        bound_u = np.ones(N * 128, dtype=np.float32)
        bound_u[pos_u] = (
            A[j_u, c_u] * mx[j_u, c_u]
            + Bs[j_u, c_u] * mx[same_idx[j_u], c_u]
            + Bd[j_u, c_u] * mx[diff_idx[j_u], c_u]
        ) * np.float32(1.02) + np.float32(1e-3)
        qs_host = np.ascontiguousarray(
            (np.float32(127.0) / bound_u).reshape(N, 128).T
        )
        scale_u = bound_u[pos_u] / np.float32(127.0)
        # per-core x slab pack [128, N*SP]
        xs_sl = []
        for k in range(NCORES):
            xsl = x16[:, :, k * SH:(k + 1) * SH, :].reshape(B, C, SP)
            rows = np.zeros((N * 128, SP), dtype=np.float16)
            rows[pos_u] = xsl[j_u, c_u]
            xs_sl.append(
                np.ascontiguousarray(
                    rows.reshape(N, 128, SP).transpose(1, 0, 2)
                ).reshape(128, N * SP)
            )
        times["host_mid"] = time.perf_counter() - t0

        rb = _get_runner(f"sparse{N}")
        _CACHE["last_apply_key"] = f"sparse{N}"
        t0 = time.perf_counter()
        xs_dev = rb.put(xs_sl)
        w_dev = rb.put_replicated(Wt_s)
        qs_dev = rb.put_replicated(qs_host)
        times["upload_b"] = time.perf_counter() - t0
        t0 = time.perf_counter()
        for attempt in range(3):
            try:
                outs_b = rb(xs_dev, w_dev, qs_dev)
                out_all = np.asarray(outs_b[0]).reshape(NCORES, 128, N * SP)
                break
            except Exception:
                if attempt == 2:
                    raise
                time.sleep(2.0)
                xs_dev = rb.put(xs_sl)
                w_dev = rb.put_replicated(Wt_s)
                qs_dev = rb.put_replicated(qs_host)
        times["exec_b"] = time.perf_counter() - t0

        t0 = time.perf_counter()
        out = x.copy()
        for k in range(NCORES):
            rows = (
                out_all[k].reshape(128, N, SP).transpose(1, 0, 2)
                .reshape(N * 128, SP)[pos_u]
            )
            deq = (rows.astype(np.float32) - np.float32(128.0)) * scale_u[:, None]
            out[j_u, c_u, k * SH:(k + 1) * SH, :] = deq.reshape(NU, SH, W)
        times["unpack"] = time.perf_counter() - t0
        _CACHE["last_times"] = times
        return out

    # ---- dense fallback: per-channel-group mixing matmuls ----------------
    # per-channel mixing matrix Wc[c, j, b]: out[b,c] = sum_j Wc[c,j,b]*x[j,c]
    Wc = np.zeros((C, B, B), dtype=np.float32)
    bi = np.arange(B)
    np.add.at(Wc, (slice(None), bi, bi), A.T)
    np.add.at(Wc, (slice(None), same_idx, bi), Bs.T)
    np.add.at(Wc, (slice(None), diff_idx, bi), Bd.T)
    # pack 4 channels per [128, 128] lhsT, interleaved-diagonal:
    # k = j*4+cc, m = b*4+cc  (channel cc of group q lives on stride-4 lanes)
    Wr = Wc.reshape(NTC, 4, B, B)
    Wblk = np.zeros((NTC, 128, 128), dtype=np.float32)
    for cc in range(4):
        Wblk[:, cc::4, cc::4] = Wr[:, cc]
    # device layout [k, q*128+m], u8 fixed-point (coefficients are in [0,1])
    Wt = np.rint(
        np.clip(
            np.ascontiguousarray(Wblk.transpose(1, 0, 2).reshape(128, NTC * 128)),
            0.0, 1.0,
        ) * np.float32(255.0)
    ).astype(np.uint8)

    # per-partition-lane output bound for the u8 fixed-point output:
    # |out[b,c,:]| <= A*max|x[b,c]| + Bs*max|x[same,c]| + Bd*max|x[diff,c]|;
    # lane p = b*4+cc covers channels {4q+cc}, so take the max over q.
    bound = A * mx + Bs * mx[same_idx] + Bd * mx[diff_idx]
    bound_lane = bound.reshape(B, NTC, 4).max(axis=1).reshape(128)
    bound_lane = bound_lane.astype(np.float32) * np.float32(1.02) + np.float32(1e-3)
    inv_lane = (np.float32(127.0) / bound_lane).astype(np.float32)
    scale_lane = (bound_lane / np.float32(127.0)).astype(np.float32)
    # B-layout: [128, NTC*SP] with row j*4+cc, col q*SP+s = x[j, q*4+cc, s]
    xq_sl = []
    for k in range(NCORES):
        sl = x16[:, :, k * SH:(k + 1) * SH, :].reshape(B, NTC, 4, SP)
        xq_sl.append(
            np.ascontiguousarray(sl.transpose(0, 2, 1, 3)).reshape(
                128, NTC * SP
            )
        )
    times["host_mid"] = time.perf_counter() - t0

    # ---- launch B: gather + mix via per-channel-group matmuls ------------
    rb = _get_runner("apply")
    _CACHE["last_apply_key"] = "apply"
    t0 = time.perf_counter()
    qs_host = inv_lane.reshape(128, 1)
    xq_dev = rb.put(xq_sl)
    w_dev = rb.put_replicated(Wt)
    qs_dev = rb.put_replicated(qs_host)
    times["upload_b"] = time.perf_counter() - t0
    t0 = time.perf_counter()
    for attempt in range(3):
        try:
            outs_b = rb(xq_dev, w_dev, qs_dev)
            out_all = np.asarray(outs_b[0]).reshape(NCORES, 128, NTC * SP)
            break
        except Exception:
            if attempt == 2:
                raise
            time.sleep(2.0)
            xq_dev = rb.put(xq_sl)
            w_dev = rb.put_replicated(Wt)
            qs_dev = rb.put_replicated(qs_host)
    times["exec_b"] = time.perf_counter() - t0

    t0 = time.perf_counter()
    out = np.empty((B, C, H, W), dtype=np.float32)
    for k in range(NCORES):
        # [128, NTC*SP] u8: row b*4+cc, col q*SP+s; dequantize per lane
        deq = (
            out_all[k].astype(np.float32) - np.float32(128.0)
        ) * scale_lane[:, None]
        blk = (
            deq
            .reshape(B, 4, NTC, SP)
            .transpose(0, 2, 1, 3)
            .reshape(B, C, SH, W)
        )
        out[:, :, k * SH:(k + 1) * SH, :] = blk
    times["unpack"] = time.perf_counter() - t0
    _CACHE["last_times"] = times
    return out

